# revision 7
# baseline (speedup 1.0000x reference)
"""HDNet 3-layer DAG-of-matmuls kernel for 8 TRN2 NeuronCores.

Math: out = concat(x0,x1) @ W0 @ W1 @ W2  (the concat/split DAG routing is
identity on the channel axis), with B=4096, C=1024, so X=[4096,2048] and
each W=[2048,2048].

Sharding: data-parallel over batch. Each core gets 512 rows of X and full
(replicated) weights. Per core:
  - A0 = X_c^T  (channels on partitions, batch on free dim), via PE transpose
  - layers 1,2: Z^T[m,:] = sum_k W[k,m].T @ A[k,:]   (activations stay
    transposed, weights are the stationary operand)
  - layer 3 swaps operands: Z3[mb,n] = sum_k A[k,mb].T @ W2[k,n], which
    yields the output in natural [batch, channel] layout directly.
All matmuls in bf16 with fp32 PSUM accumulation. Weights and X are cast
fp32->bf16 inside the (software-DGE) DMA itself.
"""

import numpy as np

import concourse.bass as bass
import concourse.bacc as bacc
import concourse.mybir as mybir
import concourse.tile as tile
from concourse.bass_utils import run_bass_kernel_spmd
from concourse.masks import make_identity

B = 4096
C = 1024
NCORES = 8
BC = B // NCORES          # 512 rows per core
K = 2 * C                 # 2048 contraction / channel dim
P = 128                   # partitions
KT = K // P               # 16 k-tiles
MT = K // P               # 16 m-tiles (output channel tiles of 128)
NBAT = BC                 # 512, batch free size (fits one PSUM bank in f32)

F32 = mybir.dt.float32
BF16 = mybir.dt.bfloat16

_built = None


def _build():
    nc = bacc.Bacc()
    x0 = nc.declare_dram_parameter("x0", [BC, C], F32, isOutput=False)
    x1 = nc.declare_dram_parameter("x1", [BC, C], F32, isOutput=False)
    ws = [
        nc.declare_dram_parameter(f"W{i}", [K, K], F32, isOutput=False)
        for i in range(3)
    ]
    out = nc.declare_dram_parameter("out", [BC, K], F32, isOutput=True)

    with tile.TileContext(nc) as tc:
        with (
            tc.tile_pool(name="xpool", bufs=4) as xpool,      # bf16 X row-tiles
            tc.tile_pool(name="wpool", bufs=2) as wpool,      # bf16 weight strips
            tc.tile_pool(name="act", bufs=2) as act,          # bf16 activation strips
            tc.tile_pool(name="outp", bufs=4) as outp,        # f32 out staging
            tc.tile_pool(name="ident", bufs=1) as identp,
            tc.tile_pool(name="psum_mm", bufs=4, space=bass.MemorySpace.PSUM) as psum_mm,
            tc.tile_pool(name="psum_tr", bufs=4, space=bass.MemorySpace.PSUM) as psum_tr,
        ):
            ident = identp.tile([P, P], BF16)
            make_identity(nc, ident[:])

            # ---- load X rows (cast f32->bf16 in DMA), PE-transpose into A0 ----
            # A strips: a[k] holds rows k*128..(k+1)*128 of the transposed
            # activation [K, NBAT] (channels on partitions, batch on free).
            a_in = []
            for k in range(KT):
                a_in.append(act.tile([P, NBAT], BF16, tag=f"a{k}", name=f"a_in{k}"))

            for r in range(BC // P):  # 4 row-tiles of X
                xb = xpool.tile([P, K], BF16, tag="xb", name=f"xb{r}")
                nc.gpsimd.dma_start(xb[:, :C], x0[r * P:(r + 1) * P, :])
                nc.gpsimd.dma_start(xb[:, C:], x1[r * P:(r + 1) * P, :])
                for k in range(KT):
                    pt = psum_tr.tile([P, P], BF16)
                    nc.tensor.transpose(pt[:], xb[:, k * P:(k + 1) * P], ident[:])
                    nc.vector.tensor_copy(a_in[k][:, r * P:(r + 1) * P], pt[:])

            # ---- weight strip loader: casting DMA straight to bf16 ----
            def load_w(layer):
                strips = []
                for k in range(KT):
                    wb = wpool.tile([P, K], BF16, tag=f"w{k}", name=f"w{layer}_{k}")
                    nc.gpsimd.dma_start(wb[:], ws[layer][k * P:(k + 1) * P, :])
                    strips.append(wb)
                return strips

            w0 = load_w(0)
            w1 = load_w(1)

            # ---- layers 1 and 2: transposed activations ----
            cur = a_in
            for layer, wstrips in ((0, w0), (1, w1)):
                nxt = []
                for m in range(MT):
                    ps = psum_mm.tile([P, NBAT], F32)
                    for k in range(KT):
                        nc.tensor.matmul(
                            ps[:],
                            wstrips[k][:, m * P:(m + 1) * P],
                            cur[k][:],
                            start=(k == 0),
                            stop=(k == KT - 1),
                        )
                    ao = act.tile([P, NBAT], BF16, tag=f"a{m}", name=f"a{layer+1}_{m}")
                    nc.vector.tensor_copy(ao[:], ps[:])
                    nxt.append(ao)
                cur = nxt
                if layer == 0:
                    w2 = load_w(2)

            # ---- layer 3: natural-layout output ----
            for mb in range(BC // P):        # 4 batch tiles
                for n in range(K // NBAT):   # 4 output-channel blocks of 512
                    ps = psum_mm.tile([P, NBAT], F32)
                    for k in range(KT):
                        nc.tensor.matmul(
                            ps[:],
                            cur[k][:, mb * P:(mb + 1) * P],
                            w2[k][:, n * NBAT:(n + 1) * NBAT],
                            start=(k == 0),
                            stop=(k == KT - 1),
                        )
                    ob = outp.tile([P, NBAT], F32)
                    nc.vector.tensor_copy(ob[:], ps[:])
                    nc.sync.dma_start(
                        out[mb * P:(mb + 1) * P, n * NBAT:(n + 1) * NBAT], ob[:]
                    )

    nc.finalize()
    return nc


def _fix_dma_waits(nc):
    """SWDGE pseudo-DMA instructions support only one sync-wait at codegen.
    Waits on the instruction's own SW ring semaphore are redundant (the ring
    is FIFO: descriptors enqueued later complete later), so drop them."""
    leftovers = []
    for f in nc.m.functions:
        for b in f.blocks:
            for i in b.instructions:
                if i.opcode != "DMACopy":
                    continue
                si = i.sync_info
                if si is None or len(si.on_wait) <= 1:
                    continue
                upd = {u.id for u in si.on_update}
                kept = [w for w in si.on_wait if w.id not in upd]
                if len(kept) != len(si.on_wait):
                    si.on_wait = kept
                if len(kept) > 1:
                    leftovers.append((i.name, [w.ant_name for w in kept]))
    if leftovers:
        raise RuntimeError(f"multi-wait DMAs remain: {leftovers[:8]}")


def _run(inputs, trace=False, **kw):
    global _built
    if _built is None:
        _built = _build()
    nc = _built
    in_maps = []
    for c in range(NCORES):
        sl = slice(c * BC, (c + 1) * BC)
        in_maps.append({
            "x0": np.ascontiguousarray(inputs["x0"][sl]),
            "x1": np.ascontiguousarray(inputs["x1"][sl]),
            "W0": inputs["W0"],
            "W1": inputs["W1"],
            "W2": inputs["W2"],
        })
    res = run_bass_kernel_spmd(nc, in_maps, list(range(NCORES)), trace=trace, **kw)
    out = np.concatenate([res.results[c]["out"] for c in range(NCORES)], axis=0)
    return out, res


def kernel(**inputs):
    out, _ = _run(inputs)
    return out


# revision 8
# speedup vs baseline: 1.0590x; 1.0590x over previous
"""HDNet 3-layer DAG-of-matmuls kernel for 8 TRN2 NeuronCores.

Math: out = concat(x0,x1) @ W0 @ W1 @ W2  (the concat/split DAG routing is
identity on the channel axis), with B=4096, C=1024, so X=[4096,2048] and
each W=[2048,2048].

Sharding: data-parallel over batch. Each core gets 512 rows of X and full
(replicated) weights. Per core:
  - A0 = X_c^T  (channels on partitions, batch on free dim), via PE transpose
  - layers 1,2: Z^T[m,:] = sum_k W[k,m].T @ A[k,:]   (activations stay
    transposed, weights are the stationary operand)
  - layer 3 swaps operands: Z3[mb,n] = sum_k A[k,mb].T @ W2[k,n], which
    yields the output in natural [batch, channel] layout directly.
All matmuls in bf16 with fp32 PSUM accumulation. Weights and X are cast
fp32->bf16 inside the (software-DGE) DMA itself.

Loop order is k-OUTER with 8 concurrent PSUM accumulation chains per group
so the in-order PE can consume weight strips as they stream from HBM
instead of stalling the first output chain on the last strip.
"""

import numpy as np

import concourse.bass as bass
import concourse.bacc as bacc
import concourse.mybir as mybir
import concourse.tile as tile
from concourse.bass_utils import run_bass_kernel_spmd
from concourse.masks import make_identity

B = 4096
C = 1024
NCORES = 8
BC = B // NCORES          # 512 rows per core
K = 2 * C                 # 2048 contraction / channel dim
P = 128                   # partitions
KT = K // P               # 16 k-tiles
MT = K // P               # 16 m-tiles (output channel tiles of 128)
NBAT = BC                 # 512, batch free size (fits one PSUM bank in f32)
G = 8                     # concurrent PSUM chains per group

F32 = mybir.dt.float32
BF16 = mybir.dt.bfloat16

_built = None


def _build():
    nc = bacc.Bacc(num_swdge_queues=4)
    x0 = nc.declare_dram_parameter("x0", [BC, C], F32, isOutput=False)
    x1 = nc.declare_dram_parameter("x1", [BC, C], F32, isOutput=False)
    ws = [
        nc.declare_dram_parameter(f"W{i}", [K, K], F32, isOutput=False)
        for i in range(3)
    ]
    out = nc.declare_dram_parameter("out", [BC, K], F32, isOutput=True)

    with tile.TileContext(nc) as tc:
        with (
            tc.tile_pool(name="xpool", bufs=4) as xpool,      # bf16 X row-tiles
            tc.tile_pool(name="wpool", bufs=2) as wpool,      # bf16 weight strips
            tc.tile_pool(name="act", bufs=2) as act,          # bf16 activation strips
            tc.tile_pool(name="outp", bufs=4) as outp,        # f32 out staging
            tc.tile_pool(name="ident", bufs=1) as identp,
            tc.tile_pool(name="psum", bufs=G, space=bass.MemorySpace.PSUM) as psum,
        ):
            ident = identp.tile([P, P], BF16)
            make_identity(nc, ident[:])

            # ---- load X rows (cast f32->bf16 in DMA), PE-transpose into A0 ----
            a_in = []
            for k in range(KT):
                a_in.append(act.tile([P, NBAT], BF16, tag=f"a{k}", name=f"a_in{k}"))

            for r in range(BC // P):  # 4 row-tiles of X
                xb = xpool.tile([P, K], BF16, tag="xb", name=f"xb{r}")
                nc.gpsimd.dma_start(xb[:, :C], x0[r * P:(r + 1) * P, :])
                nc.gpsimd.dma_start(xb[:, C:], x1[r * P:(r + 1) * P, :])
                for k in range(KT):
                    pt = psum.tile([P, P], BF16, tag="ps", name=f"pt{r}_{k}")
                    nc.tensor.transpose(pt[:], xb[:, k * P:(k + 1) * P], ident[:])
                    nc.vector.tensor_copy(a_in[k][:, r * P:(r + 1) * P], pt[:])

            # ---- weight strip loader: casting DMA straight to bf16 ----
            def load_w(layer):
                strips = []
                for k in range(KT):
                    wb = wpool.tile([P, K], BF16, tag=f"w{k}", name=f"w{layer}_{k}")
                    nc.gpsimd.dma_start(wb[:], ws[layer][k * P:(k + 1) * P, :])
                    strips.append(wb)
                return strips

            # ---- transposed-activation layer: k-outer, G chains per group ----
            def layer_t(wstrips, ain, lname):
                nxt = []
                for g0 in range(0, MT, G):
                    chains = [
                        psum.tile([P, NBAT], F32, tag="ps", name=f"ps{lname}_{g0 + j}")
                        for j in range(G)
                    ]
                    for k in range(KT):
                        for j in range(G):
                            m = g0 + j
                            nc.tensor.matmul(
                                chains[j][:],
                                wstrips[k][:, m * P:(m + 1) * P],
                                ain[k][:],
                                start=(k == 0),
                                stop=(k == KT - 1),
                            )
                    for j in range(G):
                        m = g0 + j
                        ao = act.tile(
                            [P, NBAT], BF16, tag=f"a{m}", name=f"a{lname}_{m}"
                        )
                        nc.vector.tensor_copy(ao[:], chains[j][:])
                        nxt.append(ao)
                return nxt

            w0 = load_w(0)
            a1 = layer_t(w0, a_in, "L1")
            w1 = load_w(1)       # emitted after L1 so W0 wins DMA priority
            a2 = layer_t(w1, a1, "L2")
            w2 = load_w(2)

            # ---- layer 3: natural-layout output, k-outer over (mb, n) pairs ----
            pairs = [(mb, n) for mb in range(BC // P) for n in range(K // NBAT)]
            for g0 in range(0, len(pairs), G):
                grp = pairs[g0:g0 + G]
                chains = [
                    psum.tile([P, NBAT], F32, tag="ps", name=f"psL3_{g0 + j}")
                    for j in range(len(grp))
                ]
                for k in range(KT):
                    for j, (mb, n) in enumerate(grp):
                        nc.tensor.matmul(
                            chains[j][:],
                            a2[k][:, mb * P:(mb + 1) * P],
                            w2[k][:, n * NBAT:(n + 1) * NBAT],
                            start=(k == 0),
                            stop=(k == KT - 1),
                        )
                for j, (mb, n) in enumerate(grp):
                    ob = outp.tile([P, NBAT], F32, tag="ob", name=f"ob{g0 + j}")
                    nc.vector.tensor_copy(ob[:], chains[j][:])
                    nc.sync.dma_start(
                        out[mb * P:(mb + 1) * P, n * NBAT:(n + 1) * NBAT], ob[:]
                    )

    nc.finalize()
    return nc


def _run(inputs, trace=False, **kw):
    global _built
    if _built is None:
        _built = _build()
    nc = _built
    in_maps = []
    for c in range(NCORES):
        sl = slice(c * BC, (c + 1) * BC)
        in_maps.append({
            "x0": np.ascontiguousarray(inputs["x0"][sl]),
            "x1": np.ascontiguousarray(inputs["x1"][sl]),
            "W0": inputs["W0"],
            "W1": inputs["W1"],
            "W2": inputs["W2"],
        })
    res = run_bass_kernel_spmd(nc, in_maps, list(range(NCORES)), trace=trace, **kw)
    out = np.concatenate([res.results[c]["out"] for c in range(NCORES)], axis=0)
    return out, res


def kernel(**inputs):
    out, _ = _run(inputs)
    return out
